# revision 96
# baseline (speedup 1.0000x reference)
"""Sparse (adjacency-masked) multi-head attention on 8 TRN2 NeuronCores.

Reference computation (B=2, T=2048, NX=1024, H=16, D=64):
    qkv = x @ w_attn + b_attn ; q,k,v = split(qkv)
    S = q @ k^T (per head) ; S = S*adj + NEG*(1-adj) ; P = softmax(S)
    a = (P @ v) merged-heads @ w_proj + b_proj
    out = a + q @ w_proj1 + b_proj1

Sharding: core = (batch b, head-group hg of 4 heads). Per-core partial
projections are combined with a ReduceScatter over 4-core groups.

Layout strategy (per core):
  - everything streams in bf16 (x, weights, activations); matmuls
    accumulate in f32 PSUM. bf16 halves HBM/DMA traffic (the startup is
    DMA-roofline-bound) and runs 1 cyc/row on the PE at any output width
  - host passes x^T; qT/kT computed in [dim, T] layout, v in [T, dim] layout
  - scores computed transposed: S^T[k, q] = kT_blk^T @ qT, with head PAIRS
    packed into the 128-row PE array via tile_position (K=64 each)
  - softmax: exp on ACT (no max subtraction needed; |S| <~ 30), masked
    blocks zeroed by multiplying P by the 0/1 adj block after exp; both
    heads of a pair share one [P, 2, TC] PSUM score tile and ONE exp
    instruction (ACT pays ~185ns fixed overhead per instruction), except
    the last k-block whose exps stay per-head for lower drain latency
  - 64 ones-COLUMNS appended to V: the AV matmul emits softmax sums
    pre-broadcast across PSUM rows 64-127 (matmul cost is N-cycles,
    M-free), so normalization is one DVE reciprocal + one multiply
  - block-sparse: adj classified per 128x128 block at host level (full /
    empty / partial); empty blocks are skipped entirely
  - DMA batching: HWDGE descriptor generation costs a fixed ~0.63us per
    DMA on one shared device, so x arrives as one [P, KNX, TC] tile per
    chunk (per-k pieces for chunk 0 to start the PE early), weights as
    single multi-k tiles; loads split across the SP and ACT HWDGE queues
    (a DMA holds its issuing sequencer for the whole transfer)
  - deep software pipeline: AV staggered 7 k-blocks behind scores;
    projection + bf16 ReduceScatter of chunk qc drained aggressively into
    chunk qc+1's attention stream so the collective device is free when
    the final chunk's ReduceScatter becomes ready; x tiles prefetched one
    chunk ahead; per-pair AV drained e=0 first so its normalization
    overlaps the e=1 drain
  - final-chunk tail: two tiles precompute the q@w_proj1 residual (used
    as PE filler while DVE finishes the last softmax normalization) and
    add it on DVE; the other two accumulate it in PSUM and copy out on
    the (by then idle) ACT engine, halving the serialized post-PE chain
"""
import os
import numpy as np

B, T, NX, H = 2, 2048, 1024, 16
HPC = 4            # heads per core
D = 64             # head dim
P = 128
TC = 512           # T chunk (matmul free dim)
TB = T // P        # 16 T-blocks
NTC = T // TC      # 4 T-chunks
KNX = NX // P      # 8 contraction tiles over NX
NEG = -1e9
NCORES = 8
RG = [[0, 1, 2, 3], [4, 5, 6, 7]]

_CACHE = {}


def _classify(adj):
    """Per-128x128-block classification of adj. Returns (partial dict,
    mask arrays, per-(qc,kc) spans)."""
    blk = adj.reshape(TB, P, TB, P).transpose(0, 2, 1, 3)  # [qb, kb, P, P]
    is_one = (blk == 1.0).all(axis=(2, 3))
    is_zero = (blk == 0.0).all(axis=(2, 3))

    partial = {}   # (qb, kb) -> index into mask arrays
    masks_mul = []

    def add_partial(qb, kb):
        if (qb, kb) in partial:
            return
        partial[(qb, kb)] = len(masks_mul)
        bt = blk[qb, kb].T.astype(np.float32)  # [k, q] orientation
        masks_mul.append(bt)

    # contributing k-blocks per q-chunk, and q-spans per (qc, kc)
    spans = {}     # (qc, kc) -> first q-subblock of the matmul span
    ckcs = []      # per qc: ordered list of contributing kc
    zeroed = set()  # all-zero blocks inside a span: memset, no stored mask
    for qc in range(NTC):
        qbs = range(qc * 4, qc * 4 + 4)
        kcs = [kc for kc in range(TB)
               if any(not is_zero[qb, kc] for qb in qbs)]
        assert kcs, "fully masked q-chunk not supported"
        ckcs.append(kcs)
        for i, kc in enumerate(kcs):
            if i == 0:
                q0 = qc * 4  # first kc must span the whole chunk (clears PSUM)
            else:
                q0 = min(qb for qb in qbs if not is_zero[qb, kc])
            spans[(qc, kc)] = q0
            # blocks inside the span that are not all-ones need masking
            for qb in range(q0, qc * 4 + 4):
                if is_zero[qb, kc]:
                    zeroed.add((qb, kc))
                elif not is_one[qb, kc]:
                    add_partial(qb, kc)

    npart = max(1, len(masks_mul))
    mmul = np.zeros((P, npart * P), np.float32)
    for (qb, kb), i in partial.items():
        mmul[:, i * P:(i + 1) * P] = masks_mul[i]
    return partial, mmul, spans, ckcs, zeroed


def _build(partial, npart, spans, ckcs, zeroed):
    import concourse.bass as bass
    import concourse.mybir as mybir
    import concourse.tile as tile
    from concourse import bacc

    f32 = mybir.dt.float32
    bf16 = mybir.dt.bfloat16
    EXP = mybir.ActivationFunctionType.Exp
    COPY = mybir.ActivationFunctionType.Copy
    MUL = mybir.AluOpType.mult
    ADD = mybir.AluOpType.add

    nc = bacc.Bacc(None)

    xT_p = nc.declare_dram_parameter("xT", [NX, T], bf16, isOutput=False)
    wqk_p = nc.declare_dram_parameter("wqk", [NX, 512], bf16, isOutput=False)
    wv_p = nc.declare_dram_parameter("wv", [NX, 256], bf16, isOutput=False)
    bqkT_p = nc.declare_dram_parameter("bqkT", [P, 4], f32, isOutput=False)
    bv_p = nc.declare_dram_parameter("bv", [P, 256], f32, isOutput=False)
    ones_p = nc.declare_dram_parameter("ones", [1, 512], f32, isOutput=False)
    mmul_p = nc.declare_dram_parameter("mmul", [P, npart * P], bf16, isOutput=False)
    wp_p = nc.declare_dram_parameter("wp", [256, NX], bf16, isOutput=False)
    wp1_p = nc.declare_dram_parameter("wp1", [256, NX], bf16, isOutput=False)
    out_p = nc.declare_dram_parameter("out", [NTC, P, NX], bf16, isOutput=True)

    wqk_r = wqk_p.rearrange("(ko ki) m -> ki ko m", ki=P)
    wv_r = wv_p.rearrange("(ko ki) m -> ki ko m", ki=P)
    xT_r = xT_p.rearrange("(ko ki) t -> ki ko t", ki=P)
    wp_r = wp_p.rearrange("(ko ki) m -> ki ko m", ki=P)
    wp1_r = wp1_p.rearrange("(ko ki) m -> ki ko m", ki=P)

    with tile.TileContext(nc) as tc:
        with (
            tc.tile_pool(name="persist", bufs=1) as pers,
            tc.tile_pool(name="xt", bufs=2) as xt_pool,
            tc.tile_pool(name="wk_p", bufs=9) as wk_p,
            tc.tile_pool(name="wk_s", bufs=4) as wk_s,
            tc.tile_pool(name="outp", bufs=4) as outp,
            tc.tile_pool(name="resid", bufs=8) as residp,
            tc.tile_pool(name="ps_qkv", bufs=2, space="PSUM") as ps_qkv,
            tc.tile_pool(name="ps_sc", bufs=2, space="PSUM") as ps_sc,
            tc.tile_pool(name="ps_av", bufs=2, space="PSUM") as ps_av,
            tc.tile_pool(name="dram", bufs=1, space="DRAM") as dram,
        ):
            # ---- first qkv operands before anything else, in small pieces
            # so the first matmul group starts as early as possible
            # HWDGE descriptor generation costs a FIXED ~0.63us per DMA on a
            # single shared device, so batch aggressively: x arrives as one
            # [P, KNX, TC] tile per chunk (4 pieces for chunk 0 so the PE can
            # start early), weights as single multi-k tiles. x stream on the
            # SP queue, weights on the ACT queue.
            xts0 = xt_pool.tile([P, KNX, TC], bf16, tag="xt", name="xt_0")
            wqk_sb = pers.tile([P, KNX, 512], bf16, tag="wqk", name="wqk_sb")
            wv_sb = pers.tile([P, KNX, 256], bf16, tag="wv", name="wv_sb")
            for k in range(KNX):
                nc.scalar.dma_start(wqk_sb[:, k, :], wqk_r[:, k, :])
                nc.sync.dma_start(xts0[:, k, :], xT_r[:, k, 0:TC])
                if k in (5, 7):
                    # wv halves ride inside the startup stream so the v
                    # matmuls (at ~12us) never wait on them
                    h = 0 if k == 5 else 4
                    nc.scalar.dma_start(wv_sb[:, h:h + 4, :],
                                        wv_r[:, h:h + 4, :])

            mmul_sb = [pers.tile([P, npart * P], bf16, tag="mmul",
                                 name="mmul_sb")]
            nc.scalar.dma_start(mmul_sb[0][:], mmul_p[:])
            ones_sb = pers.tile([1, 512], f32, tag="ones")
            nc.sync.dma_start(ones_sb[:], ones_p[:])
            bqkT_sb = pers.tile([P, 4], f32, tag="bqkT")
            nc.sync.dma_start(bqkT_sb[:], bqkT_p[:])
            bv_sb = pers.tile([P, 256], f32, tag="bv")
            nc.sync.dma_start(bv_sb[:], bv_p[:])
            warm_t = wk_s.tile([1, 8], f32, tag="warm")
            nc.scalar.activation(warm_t[0:1, :], ones_sb[0:1, 0:8], EXP)

            # per-(mt, tc) q/k tiles, per-(h, tc) V tiles, per-(pr, qc) a tiles
            qk_t = {(mt, tci): pers.tile([P, TC], bf16, tag=f"qk{mt}_{tci}",
                                         name=f"qk{mt}_{tci}")
                    for mt in range(4) for tci in range(NTC)}
            V_t = {(h, tci): pers.tile([P, 4, 2 * D], bf16, tag=f"V{h}_{tci}",
                                       name=f"V{h}_{tci}")
                   for h in range(HPC) for tci in range(NTC)}
            a_t = {(pr, qc): pers.tile([P, TC], bf16, tag=f"a{pr}_{qc}",
                                       name=f"a{pr}_{qc}")
                   for pr in range(2) for qc in range(NTC)}
            ones_col = pers.tile([P, 1], bf16, tag="onescol")
            nc.any.memset(ones_col[:], 1.0)
            for h in range(HPC):
                for tci in range(NTC):
                    nc.vector.tensor_copy(
                        V_t[(h, tci)][:, :, D:2 * D],
                        ones_col[:, 0:1, None].to_broadcast([P, 4, D]))

            rs_in = [dram.tile([4 * P, NX], bf16, tag=f"rsin{i}", name=f"rsin{i}")
                     for i in range(NTC)]
            rs_out = [dram.tile([P, NX], bf16, tag=f"rsout{i}", name=f"rsout{i}")
                      for i in range(NTC)]

            # attention(qc) is emitted right after qkv chunk qc+1, so ACT
            # starts exp work while PE still runs qkv matmuls
            proj_tasks = []
            wp_sb = [None]
            wp1_sb = [None]

            resid_t = {}
            PTAGS = {id(ps_qkv): "pq", id(ps_sc): "sc", id(ps_av): "av"}

            def emit_resid(t, pool=None, act_copy=False):
                # residual q @ w_proj1 for a final-chunk tile, computed early
                # so the last chunk's critical path is only the a-part
                pool = pool or ps_qkv
                for c in range(2):
                    csl = slice(c * TC, (c + 1) * TC)
                    po = pool.tile([P, TC], f32, tag=PTAGS[id(pool)],
                                   name=f"rs_po{t}_{c}")
                    for pr_ in range(2):
                        nc.tensor.matmul(
                            po[:], qk_t[(pr_, t // 4)][:, (t % 4) * P:(t % 4 + 1) * P],
                            wp1_sb[0][:, pr_, csl], start=(pr_ == 0),
                            stop=(pr_ == 1))
                    rt = residp.tile([P, TC], bf16, tag="resid",
                                     name=f"resid{t}_{c}")
                    if act_copy:
                        nc.scalar.activation(rt[:], po[:], COPY)
                    else:
                        nc.vector.tensor_copy(rt[:], po[:])
                    resid_t[(t, c)] = rt

            def emit_proj_tile(t, pool=None, act_copy=False, dma_eng=None,
                               split_dma=False):
                pool = pool or ps_qkv
                dma_eng = dma_eng or nc.scalar
                ci, cj = divmod(t, 4)
                ot = outp.tile([P, 2, TC], bf16, tag="out", name=f"ot{t}")
                for c in range(2):
                    csl = slice(c * TC, (c + 1) * TC)
                    po = pool.tile([P, TC], f32, tag=PTAGS[id(pool)],
                                   name=f"po{t}_{c}")
                    if (t, c) in resid_t:
                        for pr_ in range(2):
                            nc.tensor.matmul(
                                po[:],
                                a_t[(pr_, t // 4)][:, (t % 4) * P:(t % 4 + 1) * P],
                                wp_sb[0][:, pr_, csl], start=(pr_ == 0),
                                stop=(pr_ == 1))
                        nc.vector.tensor_tensor(ot[:, c, :], po[:],
                                                resid_t[(t, c)][:], ADD)
                    else:
                        # wp1 (q-residual) matmuls first: they have no a_t
                        # dependency, giving the PE work while DVE finishes
                        # the last normalization chain
                        for pr_ in range(2):
                            nc.tensor.matmul(
                                po[:],
                                qk_t[(pr_, t // 4)][:, (t % 4) * P:(t % 4 + 1) * P],
                                wp1_sb[0][:, pr_, csl], start=(pr_ == 0),
                                stop=False)
                        for pr_ in range(2):
                            nc.tensor.matmul(
                                po[:],
                                a_t[(pr_, t // 4)][:, (t % 4) * P:(t % 4 + 1) * P],
                                wp_sb[0][:, pr_, csl], start=False,
                                stop=(pr_ == 1))
                        if act_copy:
                            nc.scalar.activation(ot[:, c, :], po[:], COPY)
                        else:
                            nc.vector.tensor_copy(ot[:, c, :], po[:])
                    if split_dma:
                        eng = nc.sync if c == 0 else nc.scalar
                        eng.dma_start(rs_in[ci][cj * P:(cj + 1) * P, csl],
                                      ot[:, c, :])
                if not split_dma:
                    dma_eng.dma_start(rs_in[ci][cj * P:(cj + 1) * P, :], ot[:])

            def emit_task(task, pool=None, act_copy=False, dma_eng=None,
                          split_dma=False):
                if task[0] == "proj":
                    emit_proj_tile(task[1], pool, act_copy, dma_eng, split_dma)
                elif task[0] == "resid":
                    emit_resid(task[1], pool)
                else:
                    ci = task[1]
                    nc.gpsimd.collective_compute(
                        "ReduceScatter", mybir.AluOpType.add,
                        replica_groups=RG,
                        ins=[rs_in[ci].opt()], outs=[rs_out[ci].opt()])
                    if ci == NTC - 1:
                        # tail chunk: HWDGE copy is ~0.6us faster than the
                        # software-DGE path, and SP is idle by now
                        nc.sync.dma_start(out_p[ci], rs_out[ci][:])
                    else:
                        nc.gpsimd.dma_start(out_p[ci], rs_out[ci][:])

            def emit_attention(qc):
                kcs = ckcs[qc]
                last = qc == NTC - 1

                def emit_score(pr, kc, ikc):
                    q0 = spans[(qc, kc)]
                    off = q0 * P - qc * TC
                    Nn = TC - off
                    kt = qk_t[(2 + pr, kc // 4)]
                    qt = qk_t[(pr, qc)]
                    # both heads' scores land in one [P, 2, TC] PSUM pair
                    # and are exp'd by a SINGLE ACT instruction: ACT pays
                    # ~185ns fixed overhead per instruction, so merging
                    # halves the overhead on the softmax critical path
                    st2 = ps_sc.tile([P, 2, TC], f32, tag="sc",
                                     name=f"st{qc}_{pr}_{kc}")
                    for e in range(2):
                        base = 64 * e
                        nc.tensor.matmul(
                            st2[:, e, :Nn],
                            kt[base:base + 64, (kc % 4) * P:(kc % 4 + 1) * P],
                            qt[base:base + 64, off:TC],
                            start=True, stop=True, tile_position=(base, 0))
                    pt2 = wk_p.tile([P, 2, TC], bf16, tag="p",
                                    name=f"pt{qc}_{pr}_{kc}")
                    if ikc >= len(kcs) - 1:
                        # near the drain edge, per-head exps restore the
                        # lower latency the AV drain is waiting on
                        for e in range(2):
                            nc.scalar.activation(pt2[:, e, :Nn],
                                                 st2[:, e, :Nn], EXP)
                    else:
                        nc.scalar.activation(pt2[:, :, :Nn],
                                             st2[:, :, :Nn], EXP)
                    cur = []
                    for e in range(2):
                        pt = pt2[:, e, :]
                        for qb in range(q0, qc * 4 + 4):
                            key = (qb, kc)
                            c0 = qb * P - qc * TC - off
                            if key in partial:
                                i = partial[key]
                                nc.vector.tensor_tensor(
                                    pt[:, c0:c0 + P], pt[:, c0:c0 + P],
                                    mmul_sb[0][:, i * P:(i + 1) * P], MUL)
                            elif key in zeroed:
                                nc.any.memset(pt[:, c0:c0 + P], 0.0)
                        cur.append((pt, off, Nn))
                    return cur

                preview = None
                for pr in range(2):
                    av = [ps_av.tile([P, TC], f32, tag="av",
                                     name=f"av{qc}_{pr}_{ee}") for ee in range(2)]
                    pend_q = []  # [(kc, [(pt, off, Nn)] per e)] -- AV runs 6 kc late
                    for ikc, kc in enumerate(kcs):
                        if ikc >= 2 and proj_tasks:
                            emit_task(proj_tasks.pop(0))
                        if pr == 1 and ikc == 0 and preview is not None:
                            cur = preview
                        else:
                            cur = emit_score(pr, kc, ikc)
                        pend_q.append((kc, cur))
                        if len(pend_q) > 7:
                            pkc, pcur = pend_q.pop(0)
                            for e in range(2):
                                ppt, poff, pNn = pcur[e]
                                nc.tensor.matmul(
                                    av[e][:, poff:TC],
                                    V_t[(2 * pr + e, pkc // 4)][:, pkc % 4, :],
                                    ppt[:, :pNn], start=(pkc == kcs[0]),
                                    stop=False)
                    if pr == 0:
                        # pr=1's first score pair emitted BEFORE pr=0's AV
                        # drain: the drain waits on pr=0's tail exps, and
                        # this fills the PE (and frees the sc slot pinch at
                        # the pr boundary)
                        preview = emit_score(1, kcs[0], 0)
                    # drain e=0 fully, normalize it, then e=1: the e=0
                    # reciprocal+multiply on DVE overlaps the e=1 AV drain
                    # on the PE instead of trailing it.
                    # rows 64-127 of av hold the softmax sums already
                    # broadcast across 64 partitions (64 ones-columns in V):
                    # normalization is just reciprocal + multiply on DVE
                    for e in range(2):
                        for pkc, pcur in pend_q:
                            ppt, poff, pNn = pcur[e]
                            nc.tensor.matmul(av[e][:, poff:TC],
                                             V_t[(2 * pr + e, pkc // 4)][:, pkc % 4, :],
                                             ppt[:, :pNn], start=(pkc == kcs[0]),
                                             stop=(pkc == kcs[-1]))
                        rcp_t = wk_s.tile([64, TC], f32, tag="rcp",
                                          name=f"rcp{qc}_{pr}_{e}")
                        nc.vector.reciprocal(rcp_t[:], av[e][64:128, :])
                        nc.vector.tensor_tensor(
                            a_t[(pr, qc)][64 * e:64 * e + 64, :],
                            av[e][0:64, :], rcp_t[0:64, :], MUL)
                    pend_q.clear()
                if last:
                    # PE filler (no a_t dependency) while DVE runs the last
                    # normalization chain; gives tiles 12/13 a DVE-side
                    # residual while 14/15 take the 4-matmul + ACT-copy path.
                    # 14/15 drain first: their wp1 matmuls are more filler.
                    emit_resid(qc * 4)
                    emit_resid(qc * 4 + 1)
                    proj_tasks.extend([("proj", qc * 4 + 2), ("proj", qc * 4 + 3),
                                       ("proj", qc * 4), ("proj", qc * 4 + 1),
                                       ("rs", qc)])
                else:
                    proj_tasks.extend([("proj", t)
                                       for t in range(qc * 4, qc * 4 + 4)])
                    proj_tasks.append(("rs", qc))

            # ---- phase Q: qkv projections (per-k xt tiles)
            xts_next = xts0
            for tci in range(NTC):
                xts = xts_next
                tsl = slice(tci * TC, (tci + 1) * TC)
                if tci == 0:
                    # k-outer across 4 banks (borrowing idle av banks):
                    # each arriving k-tile feeds 4 matmuls, hiding the
                    # serialized startup DMA stream; k=0 split in halves so
                    # the first matmul starts on a quarter of the data
                    pqs = [ps_qkv.tile([P, TC], f32, tag="pq", name=f"pq0_{mt}")
                           for mt in range(2)]
                    pqs += [ps_av.tile([P, TC], f32, tag="av", name=f"pq0av_{mt}")
                            for mt in range(2)]
                    for k in range(KNX):
                        for mt in range(4):
                            nc.tensor.matmul(pqs[mt][:],
                                             wqk_sb[:, k, mt * P:(mt + 1) * P],
                                             xts[:, k, :], start=(k == 0),
                                             stop=(k == KNX - 1))
                    for mt in range(4):
                        nc.vector.tensor_scalar_add(qk_t[(mt, tci)][:], pqs[mt][:],
                                                    bqkT_sb[:, mt:mt + 1])
                else:
                    for mp in range(2):
                        pqs2 = [ps_qkv.tile([P, TC], f32, tag="pq",
                                            name=f"pq{tci}_{2 * mp + i}")
                                for i in range(2)]
                        for k in range(KNX):
                            for i in range(2):
                                mt = 2 * mp + i
                                nc.tensor.matmul(
                                    pqs2[i][:], wqk_sb[:, k, mt * P:(mt + 1) * P],
                                    xts[:, k, :], start=(k == 0),
                                    stop=(k == KNX - 1))
                        for i in range(2):
                            mt = 2 * mp + i
                            nc.vector.tensor_scalar_add(qk_t[(mt, tci)][:],
                                                        pqs2[i][:],
                                                        bqkT_sb[:, mt:mt + 1])
                if tci + 1 < NTC:
                    # prefetch next chunk's x as one batched DMA
                    nsl = slice((tci + 1) * TC, (tci + 2) * TC)
                    xts_next = xt_pool.tile([P, KNX, TC], bf16, tag="xt",
                                            name=f"xt_{tci + 1}")
                    nc.sync.dma_start(xts_next[:], xT_r[:, :, nsl])
                for j in range(4):
                    pv = ps_qkv.tile([P, 256], f32, tag="pq", name=f"pv{tci}_{j}")
                    for k in range(KNX):
                        nc.tensor.matmul(pv[:], xts[:, k, j * P:(j + 1) * P],
                                         wv_sb[:, k, :], start=(k == 0),
                                         stop=(k == KNX - 1))
                    for h in range(HPC):
                        nc.vector.tensor_tensor(V_t[(h, tci)][:, j, 0:D],
                                                pv[:, h * D:(h + 1) * D],
                                                bv_sb[:, h * D:(h + 1) * D], ADD)
                if tci == 2:
                    wp_sb[0] = pers.tile([P, 2, NX], bf16, tag="wp", name="wp_sb")
                    wp1_sb[0] = pers.tile([P, 2, NX], bf16, tag="wp1", name="wp1_sb")
                    nc.scalar.dma_start(wp_sb[0][:], wp_r[:])
                    nc.scalar.dma_start(wp1_sb[0][:], wp1_r[:])
                if tci >= 1:
                    emit_attention(tci - 1)
            emit_attention(NTC - 1)
            # final drain rotates across all three PSUM pools so the PE
            # never waits on the post-matmul copy of the previous po tile;
            # the two non-resid tiles copy out on ACT (idle by now)
            drain_pools = [ps_sc, ps_av, ps_qkv]
            di = 0
            while proj_tasks:
                task = proj_tasks.pop(0)
                emit_task(task, drain_pools[di % 3], act_copy=True,
                          dma_eng=(nc.sync if di % 2 == 0 else nc.scalar))
                if task[0] != "rs":
                    di += 1

    nc.finalize()
    return nc


def kernel(x, adj, w_attn, b_attn, w_proj, b_proj, w_proj1, b_proj1):
    import ml_dtypes
    from concourse.bass_utils import run_bass_kernel_spmd

    bfl = ml_dtypes.bfloat16
    x = np.asarray(x, np.float32)
    adj = np.asarray(adj, np.float32)
    w_attn = np.asarray(w_attn, np.float32)
    b_attn = np.asarray(b_attn, np.float32)
    w_proj = np.asarray(w_proj, np.float32)
    b_proj = np.asarray(b_proj, np.float32)
    w_proj1 = np.asarray(w_proj1, np.float32)
    b_proj1 = np.asarray(b_proj1, np.float32)

    partial, mmul, spans, ckcs, zeroed = _classify(adj)
    npart = max(1, len(set(partial.values())))
    key = ("g", npart, tuple(sorted(partial)), tuple(map(tuple, ckcs)),
           tuple(sorted(zeroed)))
    if key not in _CACHE:
        _CACHE[key] = _build(partial, npart, spans, ckcs, zeroed)
    nc = _CACHE[key]

    ones = np.ones((1, 512), np.float32)
    bias_total = (b_proj + b_proj1).astype(np.float32)

    in_maps = []
    for c in range(NCORES):
        b, hg = divmod(c, 4)
        cs = slice(hg * 256, (hg + 1) * 256)
        wqk = np.concatenate([w_attn[:, cs], w_attn[:, 1024:2048][:, cs]],
                             axis=1)          # [NX, 512]
        wv = w_attn[:, 2048:3072][:, cs]      # [NX, 256]
        bqkT = np.concatenate([b_attn[cs], b_attn[1024:2048][cs]]).reshape(4, P).T
        bqkT = np.ascontiguousarray(bqkT)
        bv = np.tile(b_attn[2048:3072][cs][None, :], (P, 1))
        in_maps.append({
            "xT": np.ascontiguousarray(x[b].T).astype(bfl),
            "wqk": np.ascontiguousarray(wqk).astype(bfl),
            "wv": np.ascontiguousarray(wv).astype(bfl),
            "bqkT": bqkT,
            "bv": np.ascontiguousarray(bv),
            "ones": ones,
            "mmul": mmul.astype(bfl),
            "wp": np.ascontiguousarray(w_proj[cs, :]).astype(bfl),
            "wp1": np.ascontiguousarray(w_proj1[cs, :]).astype(bfl),
        })

    trace = bool(int(os.environ.get("KERNEL_PROFILE", "0")))
    try:
        res = run_bass_kernel_spmd(nc, in_maps, core_ids=list(range(NCORES)),
                                   trace=trace)
    except Exception:
        if not trace:
            raise
        # profiling hook unavailable in this environment; rerun untraced
        res = run_bass_kernel_spmd(nc, in_maps, core_ids=list(range(NCORES)),
                                   trace=False)
    if res.exec_time_ns is not None:
        print(f"HW exec time: {res.exec_time_ns} ns")
        kernel.last_exec_time_ns = res.exec_time_ns
    if trace:
        kernel.last_results = res

    out = np.empty((B, T, NX), np.float32)
    for c in range(NCORES):
        b, r = divmod(c, 4)
        oc = np.asarray(res.results[c]["out"], np.float32)  # [4, 128, NX]
        for ci in range(NTC):
            out[b, ci * TC + r * P: ci * TC + (r + 1) * P, :] = oc[ci]
    out += bias_total[None, None, :]
    return out
